# revision 9
# baseline (speedup 1.0000x reference)
"""Local (sliding-window) self-attention Bass kernel for 8 TRN2 NeuronCores.

Problem: B=4, T=4096, C=512, H=8 heads, head_dim=64, window=15.
Sharding: 8 cores = batch(4) x seq-halves(2). Each core processes 2048 query
tokens of one batch element; its x chunk carries a 7-token halo on each side
(zero-padded at sequence edges, matching the reference's jnp.pad semantics).

Host marshalling: x arrives pre-masked and pre-transposed (feature-major,
bf16) and the weights pre-cast to bf16 and pre-split, so the device only
DMAs and runs matmuls.

Device dataflow (per core; all matmuls bf16 with fp32 PSUM):
  xT [128, 4*NKV] <- DMA (pre-masked on host)
  qT/kT = W-stationary matmuls + bias (feature-major, via ACT copy); the
    kT/qT tiles use a padded per-co stride (2176) so strided 4-block views
    stay inside one co region
  v_tok = xT-stationary matmuls + bias, token-major, packed [64 v | 1 ones]
          per head (the ones column makes AV emit the softmax denominator)
  per 4-block group g (once): B-tail scores for all 4 blocks in one matmul
    per head: lhsT = kT view [64, (4 blk, 32 k)], rhs = qT view
    [64, (4 blk, 32 q)] -> [128, 128] per head; only the block-diagonal
    (and kp<16, q'>=kp+18) entries are in-band -- everything else is exp'd
    then zeroed by one DVE multiply with a 0/1 pattern
  per 128-query block i (queries qb=128i..qb+128):
    scoresT [k, q] per head: k in [qb, qb+128), one matmul per head (even
      heads -> PSUM bank 0, odd -> bank 1: a matmul's tile_position row must
      be uniform within a bank); exp -> alpha bf16 -> one flat DVE multiply
      by the 0/1 band zeroes out-of-band entries exactly (2x DVE mode)
    AV token-major: av[q, 65] per head = alpha.T @ v_aug (den in col 64);
      B-tail adds [32k x 32q] @ v via tile_position=(32j, 96)
    reciprocal(den) * query-mask -> one DVE normalize -> avn bf16
    4 PE transposes -> attnT -> DVE int32-bitcast copy -> aT
    proj: aT-stationary matmuls; out = (bproj*mask) + psum via one DVE op
  Cross-engine pipelining: iteration i emits scores(i), out(i-2), av(i-1)
  so the PE never waits on the ACT exp or DVE normalize of the same block;
  ~4us of warm-up matmuls on `ident` keep the HAM clock gate at 8/8 while
  the input DMAs land.
"""

import math
from contextlib import ExitStack

import ml_dtypes
import numpy as np

import concourse.bacc as bacc
import concourse.bass as bass
import concourse.mybir as mybir
import concourse.tile as tile
from concourse import bass_utils

B, T, C, H, WIN = 4, 4096, 512, 8, 15
D = C // H            # 64
PAD = WIN // 2        # 7
NTOK = T // 2         # 2048 query tokens per core
NKV = 2112            # kv rows per core: 7 + 2048 + 7 = 2062, padded to 2112
STR = 2176            # padded per-co column stride of kT/qT (strided B views)
NB = NTOK // 128      # 16 query blocks
SCALE = math.log(WIN) / D
F32 = mybir.dt.float32
BF16 = mybir.dt.bfloat16
I32 = mybir.dt.int32
NWARM = 88            # PE warm-up matmuls during the input DMA window


def _mask_consts() -> dict:
    """0/1 band patterns (k on partitions, q on free) used to zero
    out-of-band alpha entries on the DVE after the exp.

    band8: A-chunk in-band (k-14 <= q <= k), replicated for 8 heads.
    band4b: B-tail supertile [(4 blk, 32 kp), 8 h x (4 blk', 32 q')]:
    valid iff blk == blk' and kp < 16 and q' >= kp + 18.
    """
    k = np.arange(128)[:, None]
    q = np.arange(128)[None, :]
    a = np.where((q >= k - 14) & (q <= k), 1.0, 0.0).astype(np.float32)
    band8 = np.tile(a, (1, 8))
    kp = np.arange(32)[:, None]
    qp = np.arange(32)[None, :]
    bsub = np.where((kp < 16) & (qp >= kp + 18), 1.0, 0.0).astype(np.float32)
    b4 = np.zeros((128, 128), np.float32)
    for blk in range(4):
        b4[blk * 32:(blk + 1) * 32, blk * 32:(blk + 1) * 32] = bsub
    band4b = np.tile(b4, (1, 8))
    return {
        "band8": np.ascontiguousarray(band8.astype(ml_dtypes.bfloat16)),
        "band4b": np.ascontiguousarray(band4b.astype(ml_dtypes.bfloat16)),
    }


def _identity() -> np.ndarray:
    return np.eye(128, dtype=ml_dtypes.bfloat16)


def build_program() -> bacc.Bacc:
    nc = bacc.Bacc("TRN2", target_bir_lowering=False, debug=False,
                   enable_asserts=False, num_devices=8)

    xtd = nc.dram_tensor("xt", [C, NKV], BF16, kind="ExternalInput").ap()
    maskd = nc.dram_tensor("mask", [NKV], F32, kind="ExternalInput").ap()
    wqd = nc.dram_tensor("wq", [C, C], BF16, kind="ExternalInput").ap()
    bqd = nc.dram_tensor("bq", [C], F32, kind="ExternalInput").ap()
    wkd = nc.dram_tensor("wk", [C, C], BF16, kind="ExternalInput").ap()
    wvd = nc.dram_tensor("wv", [C, C], BF16, kind="ExternalInput").ap()
    bkvd = nc.dram_tensor("bkv", [2 * C], F32, kind="ExternalInput").ap()
    wpd = nc.dram_tensor("wproj", [C, C], BF16, kind="ExternalInput").ap()
    bpd = nc.dram_tensor("bproj", [C], F32, kind="ExternalInput").ap()
    band8d = nc.dram_tensor("band8", [128, 1024], BF16, kind="ExternalInput").ap()
    band4bd = nc.dram_tensor("band4b", [128, 1024], BF16, kind="ExternalInput").ap()
    identd = nc.dram_tensor("ident", [128, 128], BF16, kind="ExternalInput").ap()
    outd = nc.dram_tensor("out", [NTOK, C], F32, kind="ExternalOutput").ap()

    with tile.TileContext(nc) as tc, ExitStack() as ctx:
        sb = ctx.enter_context(tc.tile_pool(name="sb", bufs=1))
        sb_a = ctx.enter_context(tc.tile_pool(name="sb_a", bufs=3))
        sb_b = ctx.enter_context(tc.tile_pool(name="sb_b", bufs=2))
        sb_o = ctx.enter_context(tc.tile_pool(name="sb_o", bufs=3))
        pp_sc = ctx.enter_context(tc.tile_pool(name="pp_sc", bufs=1, space="PSUM"))
        pp_sb = ctx.enter_context(tc.tile_pool(name="pp_sb", bufs=1, space="PSUM"))
        pp_tr = ctx.enter_context(tc.tile_pool(name="pp_tr", bufs=1, space="PSUM"))
        pp_pr = ctx.enter_context(tc.tile_pool(name="pp_pr", bufs=1, space="PSUM"))
        pp_av = ctx.enter_context(tc.tile_pool(name="pp_av", bufs=1, space="PSUM"))

        # ---- persistent SBUF tensors ----
        xT = sb.tile([128, 4 * NKV], BF16, tag="xT")     # col ci*NKV + t
        qT = sb.tile([128, 4 * STR], BF16, tag="qT")     # col co*STR + t
        kT = sb.tile([128, 4 * STR], BF16, tag="kT")     # col co*STR + t
        aT = sb.tile([128, 4 * NTOK], BF16, tag="aT")    # col ct*NTOK + q
        v_tok = [sb.tile([128, 520], BF16, tag=f"vtok{i}", name=f"vtok{i}")
                 for i in range(17)]                     # col h*65: [64 v | 1]
        band8 = sb.tile([128, 1024], BF16, tag="band8")
        band4b = sb.tile([128, 1024], BF16, tag="band4b")
        ident = sb.tile([128, 128], BF16, tag="ident")
        wq = [sb.tile([128, C], BF16, tag=f"wq{i}", name=f"wq{i}") for i in range(4)]
        wk = [sb.tile([128, C], BF16, tag=f"wk{i}", name=f"wk{i}") for i in range(4)]
        wv = [sb.tile([128, C], BF16, tag=f"wv{i}", name=f"wv{i}") for i in range(4)]
        wp = [sb.tile([128, C], BF16, tag=f"wp{i}", name=f"wp{i}") for i in range(4)]
        bq_t = sb.tile([128, 4], F32, tag="bq")       # per-partition q bias
        bk_t = sb.tile([128, 4], F32, tag="bk")       # per-partition k bias
        bvB = sb.tile([128, C], F32, tag="bvB")       # v bias bcast over partitions
        bpB = sb.tile([128, C], F32, tag="bpB")       # proj bias bcast
        mq = sb.tile([128, NB], F32, tag="mq")        # query-token mask, per block

        # ---- constants / weights / x in (DMA only; no staging casts) ----
        # Order matters: ident first (feeds the PE warm-up), then the x/wk/wq
        # chunks needed by the first compute units, then the rest.
        nc.sync.dma_start(ident[:], identd)
        for ci in range(4):
            nc.sync.dma_start(xT[:, ci * NKV:ci * NKV + 1056],
                              xtd[ci * 128:(ci + 1) * 128, 0:1056])
        for ci in range(4):
            nc.sync.dma_start(wk[ci][:], wkd[ci * 128:(ci + 1) * 128, :])
        for ci in range(4):
            nc.sync.dma_start(wq[ci][:], wqd[ci * 128:(ci + 1) * 128, :])
        nc.sync.dma_start(bq_t[:], bqd.rearrange("(a b) -> b a", b=128))
        nc.sync.dma_start(bk_t[:], bkvd[0:C].rearrange("(a b) -> b a", b=128))
        for ci in range(4):
            nc.sync.dma_start(wv[ci][:], wvd[ci * 128:(ci + 1) * 128, :])
        nc.sync.dma_start(bvB[:], bkvd[C:2 * C][None, :].broadcast_to((128, C)))
        for ci in range(4):
            nc.sync.dma_start(xT[:, ci * NKV + 1056:ci * NKV + 2112],
                              xtd[ci * 128:(ci + 1) * 128, 1056:2112])
        for ci in range(4):
            nc.sync.dma_start(wp[ci][:], wpd[ci * 128:(ci + 1) * 128, :])
        nc.sync.dma_start(bpB[:], bpd[None, :].broadcast_to((128, C)))
        nc.sync.dma_start(mq[:], maskd[PAD:PAD + NTOK].rearrange("(a b) -> b a", b=128))
        nc.sync.dma_start(band8[:], band8d)
        nc.sync.dma_start(band4b[:], band4bd)

        # ---- PE warm-up: ~4-6us of matmuls on `ident` while x streams in, so
        # the HAM clock gate is at 8/8 by the time real work starts ----
        warm = pp_pr.tile([128, 512], F32, tag="pr", name="warm")
        for w in range(NWARM):
            nc.tensor.matmul(warm[:, 0:128], ident[:], ident[:],
                             start=True, stop=True, skip_group_check=True)

        # Unit-phase PSUM tiles alternate pp_pr / pp_tr so a unit's matmuls
        # don't wait on the previous unit's ACT copy draining a single buffer.
        ucnt = [0]

        def unit_ps(nm):
            pool = pp_pr if ucnt[0] % 2 == 0 else pp_tr
            ucnt[0] += 1
            return pool.tile([128, 512], F32, tag="pr" if pool is pp_pr else "tr",
                             name=nm)

        KCH = [512, 512, 512, 512, 64]

        def emit_kT_co(ch, co):
            t0 = 512 * ch
            w = KCH[ch]
            ps = unit_ps(f"u{ch}_{co}k")
            for ci in range(4):
                nc.tensor.matmul(
                    ps[:, 0:w], wk[ci][:, co * 128:(co + 1) * 128],
                    xT[:, ci * NKV + t0:ci * NKV + t0 + w],
                    start=(ci == 0), stop=(ci == 3))
            nc.scalar.activation(kT[:, co * STR + t0:co * STR + t0 + w],
                                 ps[:, 0:w],
                                 mybir.ActivationFunctionType.Identity,
                                 bias=bk_t[:, co:co + 1])

        def emit_qT_co(ch, co):
            t0 = 512 * ch
            ps = unit_ps(f"u{ch}_{co}q")
            for ci in range(4):
                nc.tensor.matmul(
                    ps[:, 0:512], wq[ci][:, co * 128:(co + 1) * 128],
                    xT[:, ci * NKV + PAD + t0:ci * NKV + PAD + t0 + 512],
                    start=(ci == 0), stop=(ci == 3))
            nc.scalar.activation(qT[:, co * STR + t0:co * STR + t0 + 512],
                                 ps[:, 0:512],
                                 mybir.ActivationFunctionType.Identity,
                                 bias=bq_t[:, co:co + 1])

        def emit_v(t):
            r0, r1 = t * 128, min((t + 1) * 128, NKV)
            rows = r1 - r0
            ps = unit_ps(f"u{t}v")
            for ci in range(4):
                nc.tensor.matmul(
                    ps[:rows, 0:512], xT[:, ci * NKV + r0:ci * NKV + r1],
                    wv[ci][:], start=(ci == 0), stop=(ci == 3))
            vv = v_tok[t].rearrange("p (h y) -> p h y", h=8)
            nc.gpsimd.memset(vv[:, :, 64:65], 1.0)
            nc.vector.scalar_tensor_tensor(
                vv[:rows, :, 0:64],
                ps[:rows, 0:512].rearrange("p (h y) -> p h y", h=8),
                1.0,
                bvB.rearrange("p (h y) -> p h y", h=8)[:rows],
                op0=mybir.AluOpType.mult, op1=mybir.AluOpType.add)

        # ---- attention: per 128-query block, software-pipelined ----
        # PE executes in program order; iteration i emits scores(i),
        # out(i-2), av(i-1): every cross-engine input (alpha from ACT exp,
        # avn from DVE) is produced a full iteration before the PE needs it.
        alpha_t = [None] * NB
        alphb_t = [None] * (NB // 4)
        avn_t = [None] * NB
        vg_t = [None] * (NB // 4)
        kb_t = [None] * (NB // 4)
        qb_t = [None] * (NB // 4)

        def emit_bstage(g):
            # gather the strided B-tail key/query columns into compact tiles
            # (walrus requires single-free-dim matmul operands); col co*128
            # holds (4 blk, 32) lanes for that co chunk
            kb = sb_b.tile([128, 512], BF16, tag="kb", name=f"kb{g}")
            qb = sb_b.tile([128, 512], BF16, tag="qb", name=f"qb{g}")
            k0 = g * 512 + 128
            q0 = g * 512 + 96
            for co in range(4):
                nc.sync.dma_start(
                    kb[:, co * 128:(co + 1) * 128]
                      .rearrange("p (blk c) -> p blk c", blk=4),
                    kT[0:128, co * STR + k0:co * STR + k0 + 512]
                      .rearrange("p (blk c) -> p blk c", blk=4)[:, :, 0:32])
                for blk in range(4):
                    s0 = co * STR + q0 + blk * 128
                    nc.sync.dma_start(
                        qb[:, co * 128 + blk * 32:co * 128 + (blk + 1) * 32],
                        qT[0:128, s0:s0 + 32])
            kb_t[g], qb_t[g] = kb, qb

        def emit_vg(g):
            # gather rows 0:32 of v_tok[4g+1..4g+4] to partitions j*32 so the
            # AV B-tail's operands share the array row strip of alphb block j
            vg = sb_b.tile([128, 520], BF16, tag="vg", name=f"vg{g}")
            for j in range(4):
                nc.sync.dma_start(vg[j * 32:(j + 1) * 32, :],
                                  v_tok[4 * g + j + 1][0:32, :])
            vg_t[g] = vg

        def stage_scores_b(g):
            """B-tail scores for blocks 4g..4g+3, one matmul per head.

            lhsT = kT [64, (4 blk, 32 k)] (keys 128i+128..+160, stride 128),
            rhs = qT [64, (4 blk, 32 q)] (queries 128i+96..+128): the
            [128, 128] output's block-diagonal holds the real tail scores;
            off-diagonal and kp>=16 entries are zeroed post-exp by band4b.
            """
            scb = pp_sb.tile([128, 1024], F32, tag="scb", name=f"scb{g}")
            kb, qb = kb_t[g], qb_t[g]
            for b in range(2):
                for j in range(4):
                    h = 2 * j + b
                    co, hr = h // 2, (h % 2) * 64
                    nc.tensor.matmul(
                        scb[:, b * 512 + j * 128:b * 512 + (j + 1) * 128],
                        kb[hr:hr + 64, co * 128:(co + 1) * 128],
                        qb[hr:hr + 64, co * 128:(co + 1) * 128],
                        start=True, stop=True, skip_group_check=True)
            alphb = sb_b.tile([128, 1024], BF16, tag="alphb", name=f"alb{g}")
            nc.scalar.activation(alphb[:], scb[:],
                                 mybir.ActivationFunctionType.Exp, scale=SCALE)
            nc.vector.scalar_tensor_tensor(
                alphb[:], alphb[:], 1.0, band4b[:],
                op0=mybir.AluOpType.mult, op1=mybir.AluOpType.mult)
            alphb_t[g] = alphb

        def stage_scores(i):
            sc = pp_sc.tile([128, 1024], F32, tag="sc", name=f"sc{i}")
            # Heads grouped by operand partition base per PSUM bank (a
            # matmul's tile_position row must be uniform within a bank):
            # even heads (base 0) fill bank 0, odd heads (base 64) bank 1.
            for b in range(2):
                for j in range(4):
                    h = 2 * j + b
                    co, hr = h // 2, (h % 2) * 64
                    nc.tensor.matmul(
                        sc[:, b * 512 + j * 128:b * 512 + (j + 1) * 128],
                        kT[hr:hr + 64, co * STR + i * 128:co * STR + i * 128 + 128],
                        qT[hr:hr + 64, co * STR + i * 128:co * STR + (i + 1) * 128],
                        start=True, stop=True, skip_group_check=True)
            alpha = sb_a.tile([128, 1024], BF16, tag="alpha", name=f"al{i}")
            nc.scalar.activation(alpha[:], sc[:],
                                 mybir.ActivationFunctionType.Exp, scale=SCALE)
            # zero out-of-band entries exactly (band8 is 0/1; flat APs keep
            # the DVE in its 2x bf16 mode)
            nc.vector.scalar_tensor_tensor(
                alpha[:], alpha[:], 1.0, band8[:],
                op0=mybir.AluOpType.mult, op1=mybir.AluOpType.mult)
            alpha_t[i] = alpha

        def stage_av(i):
            alpha, alphb, j4 = alpha_t[i], alphb_t[i // 4], i % 4
            vg = vg_t[i // 4]
            av = pp_av.tile([128, 1024], F32, tag="av", name=f"av{i}")
            for h in range(8):
                c0 = (h // 4) * 512 + (h % 4) * 65
                ac = (h % 2) * 512 + (h // 2) * 128
                bc = ((h % 2) * 512 + (h // 2) * 128) + j4 * 32
                nc.tensor.matmul(
                    av[:, c0:c0 + 65],
                    alpha[:, ac:ac + 128],
                    v_tok[i][:, h * 65:h * 65 + 65],
                    start=True, stop=False, skip_group_check=True)
                nc.tensor.matmul(
                    av[96:128, c0:c0 + 65],
                    alphb[j4 * 32:(j4 + 1) * 32, bc:bc + 32],
                    vg[j4 * 32:(j4 + 1) * 32, h * 65:h * 65 + 65],
                    start=False, stop=True, skip_group_check=True,
                    tile_position=(j4 * 32, 96))
            avv = (av.rearrange("p (a c) -> p a c", a=2)[:, :, 0:260]
                     .rearrange("p a (h y) -> p a h y", h=4))
            rden = sb_o.tile([128, 8], F32, tag="rden", name=f"rd{i}")
            nc.vector.reciprocal(rden.rearrange("p (a h) -> p a h", a=2),
                                 avv[:, :, :, 64:65].squeeze(3))
            avn = sb_o.tile([128, 512], BF16, tag="avn", name=f"avn{i}")
            for a in range(2):
                nc.vector.scalar_tensor_tensor(
                    avn[:, a * 256:(a + 1) * 256]
                       .rearrange("p (h y) -> p h y", h=4),
                    avv[:, a:a + 1, :, 0:64].squeeze(1), mq[:, i:i + 1],
                    rden[:, a * 4:(a + 1) * 4].unsqueeze(2)
                        .broadcast_to((128, 4, 64)),
                    op0=mybir.AluOpType.mult, op1=mybir.AluOpType.mult)
            avn_t[i] = avn

        def stage_out(i):
            avn = avn_t[i]
            tr = pp_tr.tile([128, 512], BF16, tag="tr", name=f"tr{i}")
            for ct in range(4):
                nc.tensor.transpose(
                    tr[:, ct * 128:(ct + 1) * 128],
                    avn[:, ct * 128:(ct + 1) * 128],
                    ident[:])
            # bf16 copy as int32 halves the DVE element count (2x_1P)
            nc.vector.tensor_copy(
                aT.rearrange("p (a c) -> p a c", a=4)[:, :, i * 128:(i + 1) * 128]
                  .bitcast(I32),
                tr.rearrange("p (a c) -> p a c", a=4).bitcast(I32))
            pr = pp_pr.tile([128, 512], F32, tag="pr", name=f"pr{i}")
            for ct in range(4):
                nc.tensor.matmul(
                    pr[:], aT[:, ct * NTOK + i * 128:ct * NTOK + (i + 1) * 128],
                    wp[ct][:], start=(ct == 0), stop=(ct == 3))
            ot = sb_o.tile([128, C], F32, tag="ot", name=f"ot{i}")
            nc.vector.scalar_tensor_tensor(
                ot[:], bpB[:], mq[:, i:i + 1], pr[:],
                op0=mybir.AluOpType.mult, op1=mybir.AluOpType.add)
            nc.sync.dma_start(outd[i * 128:(i + 1) * 128, :], ot[:])

        # ---- prologue: kT ch0+ch1 (ch1 feeds group 0's B-tail), qT ch0,
        # v 0..2; the rest stream in as units between blocks ----
        for co in range(4):
            emit_kT_co(0, co)
        for co in range(4):
            emit_kT_co(1, co)
        for co in range(4):
            emit_qT_co(0, co)
        emit_bstage(0)
        for t in range(5):
            emit_v(t)
        emit_vg(0)

        def emit_block(i):
            if i % 4 == 0:
                stage_scores_b(i // 4)
            stage_scores(i)
            if i >= 2:
                stage_out(i - 2)
            if i >= 1:
                stage_av(i - 1)

        for g in range(4):
            if g < 2:
                units = ([lambda ch=g + 2, co=co: emit_kT_co(ch, co)
                          for co in range(4)]
                         + [lambda ch=g + 1, co=co: emit_qT_co(ch, co)
                            for co in range(4)]
                         + [lambda t=t: emit_v(t)
                            for t in range(4 * g + 5, 4 * g + 9)]
                         + [lambda g=g: emit_vg(g + 1)]
                         + [lambda g=g: emit_bstage(g + 1)])
            elif g == 2:
                units = ([lambda co=co: emit_kT_co(4, co) for co in range(4)]
                         + [lambda co=co: emit_qT_co(3, co) for co in range(4)]
                         + [lambda t=t: emit_v(t) for t in range(13, 17)]
                         + [lambda: emit_vg(3)]
                         + [lambda: emit_bstage(3)])
            else:
                units = []
            ui = 0
            nbl = 4
            for bi, i in enumerate(range(4 * g, 4 * g + 4)):
                take = (len(units) * (bi + 1)) // nbl - ui
                for u in units[ui:ui + take]:
                    u()
                ui += take
                emit_block(i)
        stage_av(NB - 1)
        stage_out(NB - 2)
        stage_out(NB - 1)

    nc.compile()
    return nc


_CACHE: dict = {}


def _get_program() -> bacc.Bacc:
    if "nc" not in _CACHE:
        _CACHE["nc"] = build_program()
    return _CACHE["nc"]


def _core_inputs(x, mask, Wq, bq, Wkv, bkv, Wproj, bproj):
    """Host-side marshalling: mask, halo-slice, transpose, cast. Returns the
    per-core input maps."""
    consts = _mask_consts()
    wq8 = np.ascontiguousarray(np.asarray(Wq, np.float32).astype(ml_dtypes.bfloat16))
    wkv = np.asarray(Wkv, np.float32)
    wk8 = np.ascontiguousarray(wkv[:, 0:C].astype(ml_dtypes.bfloat16))
    wv8 = np.ascontiguousarray(wkv[:, C:2 * C].astype(ml_dtypes.bfloat16))
    wp8 = np.ascontiguousarray(np.asarray(Wproj, np.float32).astype(ml_dtypes.bfloat16))
    shared = {
        "wq": wq8, "wk": wk8, "wv": wv8, "wproj": wp8,
        "bq": np.asarray(bq, np.float32), "bkv": np.asarray(bkv, np.float32),
        "bproj": np.asarray(bproj, np.float32),
        "ident": np.ascontiguousarray(_identity()), **consts,
    }
    if not np.all(mask == 1.0):
        x = x * mask[:, :, None]
    in_maps = []
    for core in range(8):
        b, h = divmod(core, 2)
        s = h * NTOK
        xc = np.zeros((NKV, C), np.float32)
        mc = np.zeros((NKV,), np.float32)
        lo, hi = max(0, s - PAD), min(T, s + NTOK + PAD)
        xc[lo - (s - PAD):lo - (s - PAD) + hi - lo] = x[b, lo:hi]
        mc[lo - (s - PAD):lo - (s - PAD) + hi - lo] = mask[b, lo:hi]
        in_maps.append({
            "xt": np.ascontiguousarray(xc.T.astype(ml_dtypes.bfloat16)),
            "mask": mc,
            **shared,
        })
    return in_maps


def kernel(x, mask, Wq, bq, Wkv, bkv, Wproj, bproj) -> np.ndarray:
    x = np.asarray(x, np.float32)
    mask = np.asarray(mask, np.float32)
    nc = _get_program()
    in_maps = _core_inputs(x, mask, Wq, bq, Wkv, bkv, Wproj, bproj)
    res = bass_utils.run_bass_kernel_spmd(nc, in_maps, core_ids=list(range(8)))
    out = np.empty((B, T, C), np.float32)
    for core in range(8):
        b, h = divmod(core, 2)
        out[b, h * NTOK:(h + 1) * NTOK] = res.results[core]["out"]
    return out


# revision 10
# speedup vs baseline: 1.1206x; 1.1206x over previous
"""Local (sliding-window) self-attention Bass kernel for 8 TRN2 NeuronCores.

Problem: B=4, T=4096, C=512, H=8 heads, head_dim=64, window=15.
Sharding: 8 cores = batch(4) x seq-halves(2). Each core processes 2048 query
tokens of one batch element; its x chunk carries a 7-token halo on each side
(zero-padded at sequence edges, matching the reference's jnp.pad semantics).

Host marshalling: x arrives pre-masked and pre-transposed (feature-major,
bf16) and the weights pre-cast to bf16 and pre-split, so the device only
DMAs and runs matmuls.

Device dataflow (per core; all matmuls bf16 with fp32 PSUM):
  xT [128, 4*NKV] <- DMA (pre-masked on host)
  qT/kT = W-stationary matmuls + bias (feature-major, via ACT copy); the
    kT/qT tiles use a padded per-co stride (2176) so strided 4-block views
    stay inside one co region
  v_tok = xT-stationary matmuls + bias, token-major, packed [64 v | 1 ones]
          per head (the ones column makes AV emit the softmax denominator)
  per 4-block group g (once): B-tail scores for all 4 blocks in one matmul
    per head: lhsT = kT view [64, (4 blk, 32 k)], rhs = qT view
    [64, (4 blk, 32 q)] -> [128, 128] per head; only the block-diagonal
    (and kp<16, q'>=kp+18) entries are in-band -- everything else is exp'd
    then zeroed by one DVE multiply with a 0/1 pattern
  per 128-query block i (queries qb=128i..qb+128):
    scoresT [k, q] per head: k in [qb, qb+128), one matmul per head (even
      heads -> PSUM bank 0, odd -> bank 1: a matmul's tile_position row must
      be uniform within a bank); exp -> alpha bf16 -> one flat DVE multiply
      by the 0/1 band zeroes out-of-band entries exactly (2x DVE mode)
    AV token-major: av[q, 65] per head = alpha.T @ v_aug (den in col 64);
      B-tail adds [32k x 32q] @ v via tile_position=(32j, 96)
    reciprocal(den) * query-mask -> one DVE normalize -> avn bf16
    4 PE transposes -> attnT -> DVE int32-bitcast copy -> aT
    proj: aT-stationary matmuls; out = (bproj*mask) + psum via one DVE op
  Cross-engine pipelining: iteration i emits scores(i), out(i-2), av(i-1)
  so the PE never waits on the ACT exp or DVE normalize of the same block;
  ~4us of warm-up matmuls on `ident` keep the HAM clock gate at 8/8 while
  the input DMAs land.
"""

import math
from contextlib import ExitStack

import ml_dtypes
import numpy as np

import concourse.bacc as bacc
import concourse.bass as bass
import concourse.mybir as mybir
import concourse.tile as tile
from concourse import bass_utils

B, T, C, H, WIN = 4, 4096, 512, 8, 15
D = C // H            # 64
PAD = WIN // 2        # 7
NTOK = T // 2         # 2048 query tokens per core
NKV = 2112            # kv rows per core: 7 + 2048 + 7 = 2062, padded to 2112
STR = 2176            # padded per-co column stride of kT/qT (strided B views)
NB = NTOK // 128      # 16 query blocks
SCALE = math.log(WIN) / D
F32 = mybir.dt.float32
BF16 = mybir.dt.bfloat16
I32 = mybir.dt.int32
NWARM = 120           # PE warm-up matmuls during the input DMA window


def _mask_consts() -> dict:
    """0/1 band patterns (k on partitions, q on free) used to zero
    out-of-band alpha entries on the DVE after the exp.

    band8: A-chunk in-band (k-14 <= q <= k), replicated for 8 heads.
    band4b: B-tail supertile [(4 blk, 32 kp), 8 h x (4 blk', 32 q')]:
    valid iff blk == blk' and kp < 16 and q' >= kp + 18.
    """
    k = np.arange(128)[:, None]
    q = np.arange(128)[None, :]
    a = np.where((q >= k - 14) & (q <= k), 1.0, 0.0).astype(np.float32)
    band8 = np.tile(a, (1, 8))
    kp = np.arange(32)[:, None]
    qp = np.arange(32)[None, :]
    bsub = np.where((kp < 16) & (qp >= kp + 18), 1.0, 0.0).astype(np.float32)
    b4 = np.zeros((128, 128), np.float32)
    for blk in range(4):
        b4[blk * 32:(blk + 1) * 32, blk * 32:(blk + 1) * 32] = bsub
    band4b = np.tile(b4, (1, 8))
    return {
        "band8": np.ascontiguousarray(band8.astype(ml_dtypes.bfloat16)),
        "band4b": np.ascontiguousarray(band4b.astype(ml_dtypes.bfloat16)),
    }


def _identity() -> np.ndarray:
    return np.eye(128, dtype=ml_dtypes.bfloat16)


def build_program() -> bacc.Bacc:
    nc = bacc.Bacc("TRN2", target_bir_lowering=False, debug=False,
                   enable_asserts=False, num_devices=8)

    xtd = nc.dram_tensor("xt", [C, NKV], BF16, kind="ExternalInput").ap()
    maskd = nc.dram_tensor("mask", [NKV], F32, kind="ExternalInput").ap()
    wqd = nc.dram_tensor("wq", [C, C], BF16, kind="ExternalInput").ap()
    bqd = nc.dram_tensor("bq", [C], F32, kind="ExternalInput").ap()
    wkd = nc.dram_tensor("wk", [C, C], BF16, kind="ExternalInput").ap()
    wvd = nc.dram_tensor("wv", [C, C], BF16, kind="ExternalInput").ap()
    bkvd = nc.dram_tensor("bkv", [2 * C], F32, kind="ExternalInput").ap()
    wpd = nc.dram_tensor("wproj", [C, C], BF16, kind="ExternalInput").ap()
    bpd = nc.dram_tensor("bproj", [C], F32, kind="ExternalInput").ap()
    band8d = nc.dram_tensor("band8", [128, 1024], BF16, kind="ExternalInput").ap()
    band4bd = nc.dram_tensor("band4b", [128, 1024], BF16, kind="ExternalInput").ap()
    identd = nc.dram_tensor("ident", [128, 128], BF16, kind="ExternalInput").ap()
    outd = nc.dram_tensor("out", [NTOK, C], F32, kind="ExternalOutput").ap()

    with tile.TileContext(nc) as tc, ExitStack() as ctx:
        sb = ctx.enter_context(tc.tile_pool(name="sb", bufs=1))
        sb_a = ctx.enter_context(tc.tile_pool(name="sb_a", bufs=3))
        sb_b = ctx.enter_context(tc.tile_pool(name="sb_b", bufs=2))
        sb_o = ctx.enter_context(tc.tile_pool(name="sb_o", bufs=3))
        pp_sc = ctx.enter_context(tc.tile_pool(name="pp_sc", bufs=1, space="PSUM"))
        pp_sb = ctx.enter_context(tc.tile_pool(name="pp_sb", bufs=1, space="PSUM"))
        pp_tr = ctx.enter_context(tc.tile_pool(name="pp_tr", bufs=1, space="PSUM"))
        pp_pr = ctx.enter_context(tc.tile_pool(name="pp_pr", bufs=1, space="PSUM"))
        pp_av = ctx.enter_context(tc.tile_pool(name="pp_av", bufs=1, space="PSUM"))

        # ---- persistent SBUF tensors ----
        xT = sb.tile([128, 4 * NKV], BF16, tag="xT")     # col ci*NKV + t
        qT = sb.tile([128, 4 * STR], BF16, tag="qT")     # col co*STR + t
        kT = sb.tile([128, 4 * STR], BF16, tag="kT")     # col co*STR + t
        aT = sb.tile([128, 4 * NTOK], BF16, tag="aT")    # col ct*NTOK + q
        v_tok = [sb.tile([128, 520], BF16, tag=f"vtok{i}", name=f"vtok{i}")
                 for i in range(17)]                     # col h*65: [64 v | 1]
        band8 = sb.tile([128, 1024], BF16, tag="band8")
        band4b = sb.tile([128, 1024], BF16, tag="band4b")
        ident = sb.tile([128, 128], BF16, tag="ident")
        wq = [sb.tile([128, C], BF16, tag=f"wq{i}", name=f"wq{i}") for i in range(4)]
        wk = [sb.tile([128, C], BF16, tag=f"wk{i}", name=f"wk{i}") for i in range(4)]
        wv = [sb.tile([128, C], BF16, tag=f"wv{i}", name=f"wv{i}") for i in range(4)]
        wp = [sb.tile([128, C], BF16, tag=f"wp{i}", name=f"wp{i}") for i in range(4)]
        bq_t = sb.tile([128, 4], F32, tag="bq")       # per-partition q bias
        bk_t = sb.tile([128, 4], F32, tag="bk")       # per-partition k bias
        bvB = sb.tile([128, C], F32, tag="bvB")       # v bias bcast over partitions
        bpB = sb.tile([128, C], F32, tag="bpB")       # proj bias bcast
        mq = sb.tile([128, NB], F32, tag="mq")        # query-token mask, per block

        # ---- constants / weights / x in (DMA only; no staging casts) ----
        # Order matters: ident first (feeds the PE warm-up), then the x/wk/wq
        # chunks needed by the first compute units, then the rest.
        nc.sync.dma_start(ident[:], identd)
        for ci in range(4):
            nc.sync.dma_start(xT[:, ci * NKV:ci * NKV + 1056],
                              xtd[ci * 128:(ci + 1) * 128, 0:1056])
        for ci in range(4):
            nc.sync.dma_start(wk[ci][:], wkd[ci * 128:(ci + 1) * 128, :])
        for ci in range(4):
            nc.sync.dma_start(wq[ci][:], wqd[ci * 128:(ci + 1) * 128, :])
        nc.sync.dma_start(bq_t[:], bqd.rearrange("(a b) -> b a", b=128))
        nc.sync.dma_start(bk_t[:], bkvd[0:C].rearrange("(a b) -> b a", b=128))
        for ci in range(4):
            nc.sync.dma_start(wv[ci][:], wvd[ci * 128:(ci + 1) * 128, :])
        nc.sync.dma_start(bvB[:], bkvd[C:2 * C][None, :].broadcast_to((128, C)))
        for ci in range(4):
            nc.sync.dma_start(xT[:, ci * NKV + 1056:ci * NKV + 2112],
                              xtd[ci * 128:(ci + 1) * 128, 1056:2112])
        for ci in range(4):
            nc.sync.dma_start(wp[ci][:], wpd[ci * 128:(ci + 1) * 128, :])
        nc.sync.dma_start(bpB[:], bpd[None, :].broadcast_to((128, C)))
        nc.sync.dma_start(mq[:], maskd[PAD:PAD + NTOK].rearrange("(a b) -> b a", b=128))
        nc.sync.dma_start(band8[:], band8d)
        nc.sync.dma_start(band4b[:], band4bd)

        # ---- PE warm-up: ~4-6us of matmuls on `ident` while x streams in, so
        # the HAM clock gate is at 8/8 by the time real work starts ----
        warm = pp_pr.tile([128, 512], F32, tag="pr", name="warm")
        for w in range(NWARM):
            nc.tensor.matmul(warm[:, 0:128], ident[:], ident[:],
                             start=True, stop=True, skip_group_check=True)

        # Unit-phase PSUM tiles alternate pp_pr / pp_tr so a unit's matmuls
        # don't wait on the previous unit's ACT copy draining a single buffer.
        ucnt = [0]

        def unit_ps(nm):
            pool = pp_pr if ucnt[0] % 2 == 0 else pp_tr
            ucnt[0] += 1
            return pool.tile([128, 512], F32, tag="pr" if pool is pp_pr else "tr",
                             name=nm)

        KCH = [512, 512, 512, 512, 64]

        def emit_kT_co(ch, co):
            t0 = 512 * ch
            w = KCH[ch]
            ps = unit_ps(f"u{ch}_{co}k")
            for ci in range(4):
                nc.tensor.matmul(
                    ps[:, 0:w], wk[ci][:, co * 128:(co + 1) * 128],
                    xT[:, ci * NKV + t0:ci * NKV + t0 + w],
                    start=(ci == 0), stop=(ci == 3))
            nc.scalar.activation(kT[:, co * STR + t0:co * STR + t0 + w],
                                 ps[:, 0:w],
                                 mybir.ActivationFunctionType.Identity,
                                 bias=bk_t[:, co:co + 1])

        def emit_qT_co(ch, co):
            t0 = 512 * ch
            ps = unit_ps(f"u{ch}_{co}q")
            for ci in range(4):
                nc.tensor.matmul(
                    ps[:, 0:512], wq[ci][:, co * 128:(co + 1) * 128],
                    xT[:, ci * NKV + PAD + t0:ci * NKV + PAD + t0 + 512],
                    start=(ci == 0), stop=(ci == 3))
            nc.scalar.activation(qT[:, co * STR + t0:co * STR + t0 + 512],
                                 ps[:, 0:512],
                                 mybir.ActivationFunctionType.Identity,
                                 bias=bq_t[:, co:co + 1])

        def emit_v(t):
            r0, r1 = t * 128, min((t + 1) * 128, NKV)
            rows = r1 - r0
            ps = unit_ps(f"u{t}v")
            for ci in range(4):
                nc.tensor.matmul(
                    ps[:rows, 0:512], xT[:, ci * NKV + r0:ci * NKV + r1],
                    wv[ci][:], start=(ci == 0), stop=(ci == 3))
            vv = v_tok[t].rearrange("p (h y) -> p h y", h=8)
            nc.gpsimd.memset(vv[:, :, 64:65], 1.0)
            nc.vector.scalar_tensor_tensor(
                vv[:rows, :, 0:64],
                ps[:rows, 0:512].rearrange("p (h y) -> p h y", h=8),
                1.0,
                bvB.rearrange("p (h y) -> p h y", h=8)[:rows],
                op0=mybir.AluOpType.mult, op1=mybir.AluOpType.add)

        # ---- attention: per 128-query block, software-pipelined ----
        # PE executes in program order; iteration i emits scores(i),
        # out(i-2), av(i-1): every cross-engine input (alpha from ACT exp,
        # avn from DVE) is produced a full iteration before the PE needs it.
        alpha_t = [None] * NB
        alphb_t = [None] * (NB // 4)
        avn_t = [None] * NB
        vg_t = [None] * (NB // 4)
        kb_t = [None] * (NB // 4)
        qb_t = [None] * (NB // 4)

        def emit_bstage(g):
            # gather the strided B-tail key/query columns into compact tiles
            # (walrus requires single-free-dim matmul operands); col co*128
            # holds (4 blk, 32) lanes for that co chunk
            kb = sb_b.tile([128, 512], BF16, tag="kb", name=f"kb{g}")
            qb = sb_b.tile([128, 512], BF16, tag="qb", name=f"qb{g}")
            k0 = g * 512 + 128
            q0 = g * 512 + 96
            for co in range(4):
                nc.gpsimd.dma_start(
                    kb[:, co * 128:(co + 1) * 128]
                      .rearrange("p (blk c) -> p blk c", blk=4),
                    kT[0:128, co * STR + k0:co * STR + k0 + 512]
                      .rearrange("p (blk c) -> p blk c", blk=4)[:, :, 0:32])
                nc.gpsimd.dma_start(
                    qb[:, co * 128:(co + 1) * 128]
                      .rearrange("p (blk c) -> p blk c", blk=4),
                    qT[0:128, co * STR + q0:co * STR + q0 + 512]
                      .rearrange("p (blk c) -> p blk c", blk=4)[:, :, 0:32])
            kb_t[g], qb_t[g] = kb, qb

        def emit_vg(g):
            # gather rows 0:32 of v_tok[4g+1..4g+4] to partitions j*32 so the
            # AV B-tail's operands share the array row strip of alphb block j
            vg = sb_b.tile([128, 520], BF16, tag="vg", name=f"vg{g}")
            for j in range(4):
                nc.gpsimd.dma_start(vg[j * 32:(j + 1) * 32, :],
                                    v_tok[4 * g + j + 1][0:32, :])
            vg_t[g] = vg

        def stage_scores_b(g):
            """B-tail scores for blocks 4g..4g+3, one matmul per head.

            lhsT = kT [64, (4 blk, 32 k)] (keys 128i+128..+160, stride 128),
            rhs = qT [64, (4 blk, 32 q)] (queries 128i+96..+128): the
            [128, 128] output's block-diagonal holds the real tail scores;
            off-diagonal and kp>=16 entries are zeroed post-exp by band4b.
            """
            scb = pp_sb.tile([128, 1024], F32, tag="scb", name=f"scb{g}")
            kb, qb = kb_t[g], qb_t[g]
            for b in range(2):
                for j in range(4):
                    h = 2 * j + b
                    co, hr = h // 2, (h % 2) * 64
                    nc.tensor.matmul(
                        scb[:, b * 512 + j * 128:b * 512 + (j + 1) * 128],
                        kb[hr:hr + 64, co * 128:(co + 1) * 128],
                        qb[hr:hr + 64, co * 128:(co + 1) * 128],
                        start=True, stop=True, skip_group_check=True)
            alphb = sb_b.tile([128, 1024], BF16, tag="alphb", name=f"alb{g}")
            nc.scalar.activation(alphb[:], scb[:],
                                 mybir.ActivationFunctionType.Exp, scale=SCALE)
            nc.vector.tensor_mul(alphb[:], alphb[:], band4b[:])
            alphb_t[g] = alphb

        def stage_scores(i):
            sc = pp_sc.tile([128, 1024], F32, tag="sc", name=f"sc{i}")
            # Heads grouped by operand partition base per PSUM bank (a
            # matmul's tile_position row must be uniform within a bank):
            # even heads (base 0) fill bank 0, odd heads (base 64) bank 1.
            for b in range(2):
                for j in range(4):
                    h = 2 * j + b
                    co, hr = h // 2, (h % 2) * 64
                    nc.tensor.matmul(
                        sc[:, b * 512 + j * 128:b * 512 + (j + 1) * 128],
                        kT[hr:hr + 64, co * STR + i * 128:co * STR + i * 128 + 128],
                        qT[hr:hr + 64, co * STR + i * 128:co * STR + (i + 1) * 128],
                        start=True, stop=True, skip_group_check=True)
            alpha = sb_a.tile([128, 1024], BF16, tag="alpha", name=f"al{i}")
            nc.scalar.activation(alpha[:], sc[:],
                                 mybir.ActivationFunctionType.Exp, scale=SCALE)
            # zero out-of-band entries exactly (band8 is 0/1; flat APs keep
            # the DVE in its 2x bf16 mode)
            nc.vector.tensor_mul(alpha[:], alpha[:], band8[:])
            alpha_t[i] = alpha

        def stage_av(i):
            alpha, alphb, j4 = alpha_t[i], alphb_t[i // 4], i % 4
            vg = vg_t[i // 4]
            av = pp_av.tile([128, 1024], F32, tag="av", name=f"av{i}")
            for h in range(8):
                c0 = (h // 4) * 512 + (h % 4) * 65
                ac = (h % 2) * 512 + (h // 2) * 128
                bc = ((h % 2) * 512 + (h // 2) * 128) + j4 * 32
                nc.tensor.matmul(
                    av[:, c0:c0 + 65],
                    alpha[:, ac:ac + 128],
                    v_tok[i][:, h * 65:h * 65 + 65],
                    start=True, stop=False, skip_group_check=True)
                nc.tensor.matmul(
                    av[96:128, c0:c0 + 65],
                    alphb[j4 * 32:(j4 + 1) * 32, bc:bc + 32],
                    vg[j4 * 32:(j4 + 1) * 32, h * 65:h * 65 + 65],
                    start=False, stop=True, skip_group_check=True,
                    tile_position=(j4 * 32, 96))
            avv = (av.rearrange("p (a c) -> p a c", a=2)[:, :, 0:260]
                     .rearrange("p a (h y) -> p a h y", h=4))
            rden = sb_o.tile([128, 8], F32, tag="rden", name=f"rd{i}")
            nc.vector.reciprocal(rden.rearrange("p (a h) -> p a h", a=2),
                                 avv[:, :, :, 64:65].squeeze(3))
            avn = sb_o.tile([128, 512], BF16, tag="avn", name=f"avn{i}")
            for a in range(2):
                nc.vector.scalar_tensor_tensor(
                    avn[:, a * 256:(a + 1) * 256]
                       .rearrange("p (h y) -> p h y", h=4),
                    avv[:, a:a + 1, :, 0:64].squeeze(1), mq[:, i:i + 1],
                    rden[:, a * 4:(a + 1) * 4].unsqueeze(2)
                        .broadcast_to((128, 4, 64)),
                    op0=mybir.AluOpType.mult, op1=mybir.AluOpType.mult)
            avn_t[i] = avn

        def stage_out(i):
            avn = avn_t[i]
            tr = pp_tr.tile([128, 512], BF16, tag="tr", name=f"tr{i}")
            for ct in range(4):
                nc.tensor.transpose(
                    tr[:, ct * 128:(ct + 1) * 128],
                    avn[:, ct * 128:(ct + 1) * 128],
                    ident[:])
            # bf16 copy as int32 halves the DVE element count (2x_1P)
            nc.vector.tensor_copy(
                aT.rearrange("p (a c) -> p a c", a=4)[:, :, i * 128:(i + 1) * 128]
                  .bitcast(I32),
                tr.rearrange("p (a c) -> p a c", a=4).bitcast(I32))
            pr = pp_pr.tile([128, 512], F32, tag="pr", name=f"pr{i}")
            for ct in range(4):
                nc.tensor.matmul(
                    pr[:], aT[:, ct * NTOK + i * 128:ct * NTOK + (i + 1) * 128],
                    wp[ct][:], start=(ct == 0), stop=(ct == 3))
            ot = sb_o.tile([128, C], F32, tag="ot", name=f"ot{i}")
            nc.vector.scalar_tensor_tensor(
                ot[:], bpB[:], mq[:, i:i + 1], pr[:],
                op0=mybir.AluOpType.mult, op1=mybir.AluOpType.add)
            nc.sync.dma_start(outd[i * 128:(i + 1) * 128, :], ot[:])

        # ---- prologue: kT ch0+ch1 (ch1 feeds group 0's B-tail), qT ch0,
        # v 0..2; the rest stream in as units between blocks ----
        for co in range(4):
            emit_kT_co(0, co)
        for co in range(4):
            emit_kT_co(1, co)
        for co in range(4):
            emit_qT_co(0, co)
        emit_bstage(0)
        for t in range(5):
            emit_v(t)
        emit_vg(0)

        def emit_block(i):
            if i % 4 == 0:
                stage_scores_b(i // 4)
            stage_scores(i)
            if i >= 2:
                stage_out(i - 2)
            if i >= 1:
                stage_av(i - 1)

        for g in range(4):
            if g < 2:
                units = ([lambda ch=g + 2, co=co: emit_kT_co(ch, co)
                          for co in range(4)]
                         + [lambda ch=g + 1, co=co: emit_qT_co(ch, co)
                            for co in range(4)]
                         + [lambda g=g: emit_bstage(g + 1)]
                         + [lambda t=t: emit_v(t)
                            for t in range(4 * g + 5, 4 * g + 9)]
                         + [lambda g=g: emit_vg(g + 1)])
            elif g == 2:
                units = ([lambda co=co: emit_kT_co(4, co) for co in range(4)]
                         + [lambda co=co: emit_qT_co(3, co) for co in range(4)]
                         + [lambda: emit_bstage(3)]
                         + [lambda t=t: emit_v(t) for t in range(13, 17)]
                         + [lambda: emit_vg(3)])
            else:
                units = []
            ui = 0
            nbl = 4
            for bi, i in enumerate(range(4 * g, 4 * g + 4)):
                take = (len(units) * (bi + 1)) // nbl - ui
                for u in units[ui:ui + take]:
                    u()
                ui += take
                emit_block(i)
        stage_av(NB - 1)
        stage_out(NB - 2)
        stage_out(NB - 1)

    nc.compile()
    return nc


_CACHE: dict = {}


def _get_program() -> bacc.Bacc:
    if "nc" not in _CACHE:
        _CACHE["nc"] = build_program()
    return _CACHE["nc"]


def _core_inputs(x, mask, Wq, bq, Wkv, bkv, Wproj, bproj):
    """Host-side marshalling: mask, halo-slice, transpose, cast. Returns the
    per-core input maps."""
    consts = _mask_consts()
    wq8 = np.ascontiguousarray(np.asarray(Wq, np.float32).astype(ml_dtypes.bfloat16))
    wkv = np.asarray(Wkv, np.float32)
    wk8 = np.ascontiguousarray(wkv[:, 0:C].astype(ml_dtypes.bfloat16))
    wv8 = np.ascontiguousarray(wkv[:, C:2 * C].astype(ml_dtypes.bfloat16))
    wp8 = np.ascontiguousarray(np.asarray(Wproj, np.float32).astype(ml_dtypes.bfloat16))
    shared = {
        "wq": wq8, "wk": wk8, "wv": wv8, "wproj": wp8,
        "bq": np.asarray(bq, np.float32), "bkv": np.asarray(bkv, np.float32),
        "bproj": np.asarray(bproj, np.float32),
        "ident": np.ascontiguousarray(_identity()), **consts,
    }
    if not np.all(mask == 1.0):
        x = x * mask[:, :, None]
    in_maps = []
    for core in range(8):
        b, h = divmod(core, 2)
        s = h * NTOK
        xc = np.zeros((NKV, C), np.float32)
        mc = np.zeros((NKV,), np.float32)
        lo, hi = max(0, s - PAD), min(T, s + NTOK + PAD)
        xc[lo - (s - PAD):lo - (s - PAD) + hi - lo] = x[b, lo:hi]
        mc[lo - (s - PAD):lo - (s - PAD) + hi - lo] = mask[b, lo:hi]
        in_maps.append({
            "xt": np.ascontiguousarray(xc.T.astype(ml_dtypes.bfloat16)),
            "mask": mc,
            **shared,
        })
    return in_maps


def kernel(x, mask, Wq, bq, Wkv, bkv, Wproj, bproj) -> np.ndarray:
    x = np.asarray(x, np.float32)
    mask = np.asarray(mask, np.float32)
    nc = _get_program()
    in_maps = _core_inputs(x, mask, Wq, bq, Wkv, bkv, Wproj, bproj)
    res = bass_utils.run_bass_kernel_spmd(nc, in_maps, core_ids=list(range(8)))
    out = np.empty((B, T, C), np.float32)
    for core in range(8):
        b, h = divmod(core, 2)
        out[b, h * NTOK:(h + 1) * NTOK] = res.results[core]["out"]
    return out
